# revision 6
# baseline (speedup 1.0000x reference)
"""4-layer GCN (PyG GCNConv-style) + final MLP layer on 8 Trainium2 NeuronCores.

Strategy (graph/data parallel, per the sharding hint):
 - Nodes sharded contiguously across 8 cores (6250 each).
 - Host prep (graph structure only): degrees/normalization from edge_index,
   self-loop edges appended, edges sorted by destination, packed into
   128-edge chunks per 128-node destination tile with a chunk schedule
   shared by all cores (SPMD: one program).
 - Per layer each core computes its shard of y = dinv * (h @ W) (the
   pre-normalized message table), AllGathers the full table into local HBM,
   then aggregates: gathers source rows y[src] with the int16 dma_gather
   DMA (table split in two <=32768-row halves to fit int16 indexing; chunks
   are homogeneous per half) and accumulates them into PSUM using one-hot
   selection-matrix matmuls on TensorE. Selection values carry dinv_dst and
   the bias enters as one identity-matmul of a broadcast bias tile, so PSUM
   directly holds dinv*(agg) + b; ELU follows on ACT/DVE.
 - Final layer S = elu(h @ Wm2 + bm2) on each core's shard (the reference's
   loop bug makes Wm1/bm1 dead).
"""

import sys

sys.path.insert(0, "/opt/trn_rl_repo")

import numpy as np

import concourse.bass as bass
import concourse.mybir as mybir
import concourse.bacc as bacc
import concourse.tile as tile
from concourse import bass_utils

F32 = mybir.dt.float32
I16 = mybir.dt.int16
AF = mybir.ActivationFunctionType
OP = mybir.AluOpType

N_CORES = 8
H = 128
C_OUT = 10
G = 32  # chunks per dma_gather call (rows per call = 128*G)
HALF = 32768  # int16 index limit splits the table into two halves


def _host_prep(edge_index, n_nodes, n_cores):
    """Graph preprocessing -> shared chunk schedule + per-core index arrays."""
    shard = n_nodes // n_cores
    tiles = (shard + 127) // 128
    src = np.asarray(edge_index[0]).astype(np.int64)
    dst = np.asarray(edge_index[1]).astype(np.int64)
    deg = np.bincount(dst, minlength=n_nodes).astype(np.float64) + 1.0
    dinv = (1.0 / np.sqrt(deg)).astype(np.float32)

    arange = np.arange(n_nodes, dtype=np.int64)
    s_all = np.concatenate([src, arange])
    d_all = np.concatenate([dst, arange])
    # sort by (dst, src-half) so each (tile, half) span is contiguous
    half = (s_all >= HALF).astype(np.int64)
    order = np.lexsort((half, d_all))
    s_all = s_all[order]
    d_all = d_all[order]
    h_all = half[order]
    key = d_all * 2 + h_all  # search key for (dst, half) spans

    core_of = d_all // shard
    tloc = (d_all - core_of * shard) // 128
    cntA = np.zeros((n_cores, tiles), np.int64)
    cntB = np.zeros((n_cores, tiles), np.int64)
    np.add.at(cntA, (core_of[h_all == 0], tloc[h_all == 0]), 1)
    np.add.at(cntB, (core_of[h_all == 1], tloc[h_all == 1]), 1)
    K_A = (-(-cntA.max(axis=0) // 128)).astype(np.int64)
    K_B = (-(-cntB.max(axis=0) // 128)).astype(np.int64)
    CA, CB = int(K_A.sum()), int(K_B.sum())
    offA = np.concatenate([[0], np.cumsum(K_A)])
    offB = np.concatenate([[0], np.cumsum(K_B)])

    per_core = []
    for c in range(n_cores):
        arr = {
            "idxA": np.zeros((16, CA * 8), np.int16),
            "idxB": np.zeros((16, CB * 8), np.int16),
            "dstlocA": np.full((128, max(CA, 1)), -1.0, np.float32),
            "dstlocB": np.full((128, max(CB, 1)), -1.0, np.float32),
            "dinvdA": np.zeros((128, max(CA, 1)), np.float32),
            "dinvdB": np.zeros((128, max(CB, 1)), np.float32),
        }
        for t in range(tiles):
            lo = c * shard + t * 128
            hi = min(c * shard + (t + 1) * 128, (c + 1) * shard)
            for hbit, nm, off in ((0, "A", offA), (1, "B", offB)):
                a = np.searchsorted(key, lo * 2 + hbit, side="left")
                b = np.searchsorted(key, (hi - 1) * 2 + hbit, side="right")
                # span includes both halves; filter
                ss = s_all[a:b]
                dd = d_all[a:b]
                hh = h_all[a:b]
                m = hh == hbit
                ss, dd = ss[m], dd[m]
                n = ss.shape[0]
                if n == 0:
                    continue
                e = np.arange(n)
                p = e % 128
                ch = e // 128 + off[t]
                arr["idx" + nm][p % 16, ch * 8 + p // 16] = (
                    ss - hbit * HALF
                ).astype(np.int16)
                arr["dstloc" + nm][p, ch] = (dd - lo).astype(np.float32)
                arr["dinvd" + nm][p, ch] = dinv[dd]
        # replicate idx across the 8 16-partition groups
        arr["idxA"] = np.tile(arr["idxA"], (8, 1))
        arr["idxB"] = np.tile(arr["idxB"], (8, 1))
        per_core.append(arr)
    return K_A, K_B, per_core, dinv, shard, tiles, CA, CB


def _build_program(K_A, K_B, n_nodes, n_cores, CA, CB, tiles, shard):
    nc = bacc.Bacc(
        "TRN2",
        target_bir_lowering=False,
        debug=False,
        num_devices=n_cores,
        num_swdge_queues=4,
    )
    TP = tiles * 128

    xT_in = nc.dram_tensor("xT", [128, TP], F32, kind="ExternalInput")
    W_in = nc.dram_tensor("W", [4, 128, H], F32, kind="ExternalInput")
    Wm2_in = nc.dram_tensor("Wm2", [128, C_OUT], F32, kind="ExternalInput")
    bb_in = nc.dram_tensor("bb", [4, 128, H], F32, kind="ExternalInput")
    bm2_in = nc.dram_tensor("bm2", [128, C_OUT], F32, kind="ExternalInput")
    iota_in = nc.dram_tensor("iota", [128, 128], F32, kind="ExternalInput")
    ident_in = nc.dram_tensor("ident", [128, 128], F32, kind="ExternalInput")
    dinv_in = nc.dram_tensor("dinvc", [128, tiles], F32, kind="ExternalInput")
    idxA_in = nc.dram_tensor("idxA", [128, CA * 8], I16, kind="ExternalInput")
    idxB_in = nc.dram_tensor("idxB", [128, CB * 8], I16, kind="ExternalInput")
    dstlocA_in = nc.dram_tensor("dstlocA", [128, max(CA, 1)], F32, kind="ExternalInput")
    dstlocB_in = nc.dram_tensor("dstlocB", [128, max(CB, 1)], F32, kind="ExternalInput")
    dinvdA_in = nc.dram_tensor("dinvdA", [128, max(CA, 1)], F32, kind="ExternalInput")
    dinvdB_in = nc.dram_tensor("dinvdB", [128, max(CB, 1)], F32, kind="ExternalInput")

    h_out = nc.dram_tensor("h_out", [shard, H], F32, kind="ExternalOutput")
    s_out = nc.dram_tensor("s_out", [shard, C_OUT], F32, kind="ExternalOutput")

    last_rows = shard - 128 * (tiles - 1)

    with tile.TileContext(nc) as tc:
        with (
            tc.tile_pool(name="const", bufs=1) as constp,
            tc.tile_pool(name="dram", bufs=1, space="DRAM") as dramp,
            tc.tile_pool(name="hbuf", bufs=2) as hbufp,
            tc.tile_pool(name="hT", bufs=3) as hTp,
            tc.tile_pool(name="ysb", bufs=3) as ysbp,
            tc.tile_pool(name="sel", bufs=4) as selp,
            tc.tile_pool(name="gath", bufs=2) as gathp,
            tc.tile_pool(name="epi", bufs=3) as epip,
            tc.tile_pool(name="ptp", bufs=2, space="PSUM") as ptp,
            tc.tile_pool(name="pxw", bufs=2, space="PSUM") as pxw,
            tc.tile_pool(name="pagg", bufs=3, space="PSUM") as pagg,
            tc.tile_pool(name="ps", bufs=1, space="PSUM") as psp,
        ):
            # ---- constants ----
            W_sb, bb_sb = [], []
            for l in range(4):
                w = constp.tile([128, H], F32, name=f"W{l}")
                nc.sync.dma_start(out=w[:], in_=W_in[l])
                W_sb.append(w)
                b = constp.tile([128, H], F32, name=f"bb{l}")
                nc.sync.dma_start(out=b[:], in_=bb_in[l])
                bb_sb.append(b)
            Wm2_sb = constp.tile([128, C_OUT], F32)
            nc.sync.dma_start(out=Wm2_sb[:], in_=Wm2_in[:])
            bm2_sb = constp.tile([128, C_OUT], F32)
            nc.sync.dma_start(out=bm2_sb[:], in_=bm2_in[:])
            iota_sb = constp.tile([128, 128], F32)
            nc.sync.dma_start(out=iota_sb[:], in_=iota_in[:])
            ident_sb = constp.tile([128, 128], F32)
            nc.sync.dma_start(out=ident_sb[:], in_=ident_in[:])
            dinv_sb = constp.tile([128, tiles], F32)
            nc.sync.dma_start(out=dinv_sb[:], in_=dinv_in[:])
            idxA_sb = constp.tile([128, CA * 8], I16)
            nc.sync.dma_start(out=idxA_sb[:], in_=idxA_in[:])
            idxB_sb = constp.tile([128, CB * 8], I16)
            nc.sync.dma_start(out=idxB_sb[:], in_=idxB_in[:])
            dstlocA_sb = constp.tile([128, max(CA, 1)], F32)
            nc.sync.dma_start(out=dstlocA_sb[:], in_=dstlocA_in[:])
            dstlocB_sb = constp.tile([128, max(CB, 1)], F32)
            nc.sync.dma_start(out=dstlocB_sb[:], in_=dstlocB_in[:])
            dinvdA_sb = constp.tile([128, max(CA, 1)], F32)
            nc.sync.dma_start(out=dinvdA_sb[:], in_=dinvdA_in[:])
            dinvdB_sb = constp.tile([128, max(CB, 1)], F32)
            nc.sync.dma_start(out=dinvdB_sb[:], in_=dinvdB_in[:])

            y_stages = [
                dramp.tile([shard, H], F32, name=f"y_stage{l}") for l in range(4)
            ]
            y_fulls = [
                dramp.tile(
                    [n_cores * shard, H], F32, addr_space="Shared", name=f"y_full{l}"
                )
                for l in range(4)
            ]

            def elu_chain(zpsum, out_ap, w):
                """out = elu(z); z read from PSUM. w = valid free width."""
                u = epip.tile([128, w], F32, tag=f"u{w}")
                nc.scalar.activation(out=u[:], in_=zpsum, func=AF.Relu, scale=-1.0)
                e = epip.tile([128, w], F32, tag=f"e{w}")
                nc.scalar.activation(out=e[:], in_=u[:], func=AF.Exp, scale=-1.0)
                r = epip.tile([128, w], F32, tag=f"r{w}")
                nc.vector.tensor_scalar(
                    out=r[:],
                    in0=zpsum,
                    scalar1=0.0,
                    scalar2=1.0,
                    op0=OP.max,
                    op1=OP.subtract,
                )
                nc.vector.tensor_tensor(out=out_ap, in0=e[:], in1=r[:], op=OP.add)

            h_cur = None
            for l in range(4):
                y_stage = y_stages[l]
                y_full = y_fulls[l]
                # ---- phase A: y_shard = dinv * (h @ W_l) -> y_stage ----
                for t in range(tiles):
                    if l == 0:
                        hT = hTp.tile([128, 128], F32, tag="hT")
                        nc.sync.dma_start(
                            out=hT[:], in_=xT_in[:, t * 128 : (t + 1) * 128]
                        )
                    else:
                        tp = ptp.tile([128, 128], F32, tag="tp")
                        nc.tensor.transpose(
                            out=tp[:],
                            in_=h_cur[:, t * 128 : (t + 1) * 128],
                            identity=ident_sb[:],
                        )
                        hT = hTp.tile([128, 128], F32, tag="hT")
                        nc.scalar.copy(out=hT[:], in_=tp[:])
                    xw = pxw.tile([128, H], F32, tag="xw")
                    nc.tensor.matmul(
                        out=xw[:], lhsT=hT[:], rhs=W_sb[l][:], start=True, stop=True
                    )
                    ysb = ysbp.tile([128, H], F32, tag="ysb")
                    nc.vector.tensor_scalar(
                        out=ysb[:],
                        in0=xw[:],
                        scalar1=dinv_sb[:, t : t + 1],
                        scalar2=None,
                        op0=OP.mult,
                    )
                    rows = 128 if t < tiles - 1 else last_rows
                    nc.sync.dma_start(
                        out=y_stage[t * 128 : t * 128 + rows, :], in_=ysb[:rows, :]
                    )

                # ---- phase B: AllGather the message table ----
                nc.gpsimd.collective_compute(
                    "AllGather",
                    OP.bypass,
                    replica_groups=[list(range(n_cores))],
                    ins=[y_stage[:]],
                    outs=[y_full[:]],
                )

                # ---- phase C: aggregate per destination tile ----
                h_next = hbufp.tile([128, TP], F32, tag="hbuf")
                cur = {"A": None, "B": None}
                qs = {"A": 0, "B": 0}
                call_no = l * 1000  # distinct rotation across layers
                srcs = {
                    "A": (y_full[0:HALF, :], idxA_sb, dstlocA_sb, dinvdA_sb, CA),
                    "B": (
                        y_full[HALF : n_cores * shard, :],
                        idxB_sb,
                        dstlocB_sb,
                        dinvdB_sb,
                        CB,
                    ),
                }

                for t in range(tiles):
                    agg = pagg.tile([128, H], F32, tag="agg")
                    nc.tensor.matmul(
                        out=agg[:],
                        lhsT=ident_sb[:],
                        rhs=bb_sb[l][:],
                        start=True,
                        stop=False,
                    )
                    work = [("A", j) for j in range(int(K_A[t]))] + [
                        ("B", j) for j in range(int(K_B[t]))
                    ]
                    for wi, (nm, j) in enumerate(work):
                        table_ap, idx_sb, dl_sb, dv_sb, Ctot = srcs[nm]
                        q = qs[nm]
                        g, slot = divmod(q, G)
                        if slot == 0:
                            w = min(G, Ctot - g * G)
                            gt = gathp.tile([128, G * 128], F32, tag="g" + nm)
                            nc.gpsimd.dma_gather(
                                gt[:, : w * 128].rearrange("p (c e) -> p c e", e=128),
                                table_ap,
                                idx_sb[:, g * G * 8 : (g * G + w) * 8],
                                w * 128,
                                w * 128,
                                H,
                                elem_step=H,
                                single_packet=False,
                                queue_num=call_no % 4,
                            )
                            call_no += 1
                            cur[nm] = gt
                        sel = selp.tile([128, 128], F32, tag="sel")
                        nc.vector.tensor_scalar(
                            out=sel[:],
                            in0=iota_sb[:],
                            scalar1=dl_sb[:, q : q + 1],
                            scalar2=dv_sb[:, q : q + 1],
                            op0=OP.is_equal,
                            op1=OP.mult,
                        )
                        nc.tensor.matmul(
                            out=agg[:],
                            lhsT=sel[:],
                            rhs=cur[nm][:, slot * 128 : (slot + 1) * 128],
                            start=False,
                            stop=(wi == len(work) - 1),
                        )
                        qs[nm] = q + 1
                    assert work, "every tile must schedule at least one chunk"
                    elu_chain(agg[:], h_next[:, t * 128 : (t + 1) * 128], 128)
                h_cur = h_next

            # ---- phase D: outputs h and S = elu(h @ Wm2 + bm2) ----
            for t in range(tiles):
                rows = 128 if t < tiles - 1 else last_rows
                nc.sync.dma_start(
                    out=h_out[t * 128 : t * 128 + rows, :],
                    in_=h_cur[:rows, t * 128 : (t + 1) * 128],
                )
                tp = ptp.tile([128, 128], F32, tag="tp")
                nc.tensor.transpose(
                    out=tp[:],
                    in_=h_cur[:, t * 128 : (t + 1) * 128],
                    identity=ident_sb[:],
                )
                hT = hTp.tile([128, 128], F32, tag="hT")
                nc.scalar.copy(out=hT[:], in_=tp[:])
                sp = psp.tile([128, C_OUT], F32, tag="sp")
                nc.tensor.matmul(
                    out=sp[:], lhsT=ident_sb[:], rhs=bm2_sb[:], start=True, stop=False
                )
                nc.tensor.matmul(
                    out=sp[:], lhsT=hT[:], rhs=Wm2_sb[:], start=False, stop=True
                )
                st = epip.tile([128, C_OUT], F32, tag="st")
                elu_chain(sp[:], st[:], C_OUT)
                nc.sync.dma_start(
                    out=s_out[t * 128 : t * 128 + rows, :], in_=st[:rows, :]
                )

    nc.compile()
    return nc


def prepare(x, edge_index, W1, b1, W2, b2, W3, b3, W4, b4, Wm1, bm1, Wm2, bm2):
    """Host prep + program build. Returns (nc, in_maps, shard)."""
    x = np.asarray(x, dtype=np.float32)
    n_nodes = x.shape[0]
    K_A, K_B, per_core, dinv, shard, tiles, CA, CB = _host_prep(
        edge_index, n_nodes, N_CORES
    )
    nc = _build_program(K_A, K_B, n_nodes, N_CORES, CA, CB, tiles, shard)

    W = np.stack([np.asarray(w, np.float32) for w in (W1, W2, W3, W4)], axis=0)
    bb = np.stack(
        [np.tile(np.asarray(b, np.float32)[None, :], (128, 1)) for b in (b1, b2, b3, b4)],
        axis=0,
    )
    bm2_b = np.tile(np.asarray(bm2, np.float32)[None, :], (128, 1))
    iota = np.tile(np.arange(128, dtype=np.float32)[None, :], (128, 1))
    ident = np.eye(128, dtype=np.float32)

    TP = tiles * 128
    in_maps = []
    for c in range(N_CORES):
        xs = x[c * shard : (c + 1) * shard]
        xT = np.zeros((128, TP), np.float32)
        xT[:, : xs.shape[0]] = xs.T
        dc = np.zeros((128, tiles), np.float32)
        dv = dinv[c * shard : (c + 1) * shard]
        for t in range(tiles):
            seg = dv[t * 128 : (t + 1) * 128]
            dc[: seg.shape[0], t] = seg
        pc = per_core[c]
        in_maps.append(
            {
                "xT": xT,
                "W": W,
                "Wm2": np.asarray(Wm2, np.float32),
                "bb": bb,
                "bm2": bm2_b,
                "iota": iota,
                "ident": ident,
                "dinvc": dc,
                "idxA": pc["idxA"],
                "idxB": pc["idxB"],
                "dstlocA": pc["dstlocA"],
                "dstlocB": pc["dstlocB"],
                "dinvdA": pc["dinvdA"],
                "dinvdB": pc["dinvdB"],
            }
        )
    return nc, in_maps, shard


def kernel(**inputs):
    nc, in_maps, shard = prepare(**inputs)
    res = bass_utils.run_bass_kernel_spmd(nc, in_maps, core_ids=list(range(N_CORES)))
    h = np.concatenate([r["h_out"] for r in res.results], axis=0)
    S = np.concatenate([r["s_out"] for r in res.results], axis=0)
    return h, S
